# revision 31
# baseline (speedup 1.0000x reference)
"""ExLlama transformer layer (GPTQ int4) on 8 TRN2 NeuronCores, tensor-parallel.

Self-contained: hardcodes shapes from the problem spec.
  B=1, S=2048, HID=4096, INTER=11008, HEADS=32, HD=128, GS=128.

Sharding (SPMD, identical program per core, per-core data slices):
  - q/k/v column-sharded (4 heads per core), attention head-parallel
  - o row-sharded -> partial sums -> fp16 AllReduce per seq-half -> h2
  - gate/up column-sharded over padded INTER (8*1408=11264, zero-padded)
  - down row-sharded -> partials (+h2/8) -> fp16 ReduceScatter per
    feature-half; host reassembles the permuted feature order.

The attention super-phase runs per sequence-half (x1 only half-resident in
SBUF), so the first half's AllReduce overlaps the second half's compute.
h2 = h + ar is built SBUF-resident (bf16) fused into the MLP rms pass; the
residual rides through the down partials as +h2/8.
"""
import sys

sys.path.insert(0, "/opt/trn_rl_repo")

import numpy as np

S = 2048
HID = 4096
HD = 128
GS = 128
INTER = 11008
NCORES = 8
IPC = 1408                      # padded inter features per core
IPAD = IPC * NCORES             # 11264
NKT = HID // 128                # 32 k-tiles over HID
NIT = IPC // 128                # 11 k-tiles over per-core inter
OPC = HID // NCORES             # 512 out features per core (qkv), 4 heads
NHC = OPC // HD                 # 4 heads per core
CHUNK = 512
NCHUNK = S // CHUNK             # 4
NST = S // 128                  # 16 s-tiles
HKT = NKT // 2                  # 16 k-tiles per feature half
SH = S // 2                     # 1024, seq half
SCALE = 1.0 / float(np.sqrt(HD))
EPS = 1e-6
NEG = -30000.0

_BUILD_CACHE = {}


def _build():
    import concourse.bacc as bacc
    import concourse.mybir as mybir
    import concourse.tile as tile
    import ml_dtypes

    dt = mybir.dt
    F32, F16, BF16, I32 = dt.float32, dt.float16, dt.bfloat16, dt.int32
    Alu = mybir.AluOpType
    Act = mybir.ActivationFunctionType

    nc = bacc.Bacc("TRN2", target_bir_lowering=False, num_devices=NCORES)

    # ---------------- external I/O ----------------
    hT_d = nc.dram_tensor("hT", [HID, S], F32, kind="ExternalInput")
    cosT_d = nc.dram_tensor("cosT", [HD, S], F16, kind="ExternalInput")
    sinfT_d = nc.dram_tensor("sinfT", [HD, S], F16, kind="ExternalInput")
    ln1_d = nc.dram_tensor("ln1", [HID], F32, kind="ExternalInput")
    ln2_d = nc.dram_tensor("ln2", [HID], F32, kind="ExternalInput")
    qw_qkv_d = nc.dram_tensor("qw_qkv", [12 * 16, NKT * 128], I32, kind="ExternalInput")
    sc_qkv_d = nc.dram_tensor("sc_qkv", [12 * NKT, 128], F16, kind="ExternalInput")
    nz_qkv_d = nc.dram_tensor("nz_qkv", [32, 3 * OPC], F16, kind="ExternalInput")
    qw_o_d = nc.dram_tensor("qw_o", [NKT * 16, NHC * 128], I32, kind="ExternalInput")
    sc_o_d = nc.dram_tensor("sc_o", [NKT * NHC, 128], F16, kind="ExternalInput")
    nz_o_d = nc.dram_tensor("nz_o", [OPC // GS, HID], F16, kind="ExternalInput")
    qw_gu_d = nc.dram_tensor("qw_gu", [2 * NIT * 16, NKT * 128], I32, kind="ExternalInput")
    sc_gu_d = nc.dram_tensor("sc_gu", [2 * NIT * NKT, 128], BF16, kind="ExternalInput")
    nz_gu_d = nc.dram_tensor("nz_gu", [32, 2 * IPC], BF16, kind="ExternalInput")
    qw_dn_d = nc.dram_tensor("qw_dn", [NKT * 16, NIT * 128], I32, kind="ExternalInput")
    sc_dn_d = nc.dram_tensor("sc_dn", [NKT * NIT, 128], BF16, kind="ExternalInput")
    nz_dn_d = nc.dram_tensor("nz_dn", [NIT, HID], BF16, kind="ExternalInput")

    outA_d = nc.dram_tensor("outA", [OPC // 2, S], F16, kind="ExternalOutput")
    outB_d = nc.dram_tensor("outB", [OPC // 2, S], F16, kind="ExternalOutput")

    # ---------------- inline constants ----------------
    p = np.arange(128)
    sh_c = nc.inline_tensor(((p % 8) * 4).astype(np.int32)[:, None], name="shc")
    id16_c = nc.inline_tensor(np.eye(128, dtype=np.float16), name="id16")
    onesbf_c = nc.inline_tensor(np.ones((128, 1), ml_dtypes.bfloat16),
                                name="onesbf")
    # wide selector: eslw[:, j] = 1 iff j == 31; slice [31-t : 63-t] gives a
    # [128, 32] tile whose column t is all-ones (rowsum-per-group lhsT).
    eslw = np.zeros((128, 63), np.float16)
    eslw[:, 31] = 1.0
    esw16_c = nc.inline_tensor(eslw, name="esw16")
    eswbf_c = nc.inline_tensor(eslw.astype(ml_dtypes.bfloat16), name="eswbf")
    # wide causal mask, S^T layout: mkw[k, j] with qq = j-384: 0 if qq >= k
    # else NEG.  Slice [384-128*dd : 896-128*dd] = mask for diag offset dd.
    j = np.arange(896)
    mkw = np.where((j[None, :] - 384) >= p[:, None], 0.0, NEG)
    mask_c = nc.inline_tensor(mkw.astype(ml_dtypes.bfloat16), name="maskc")

    # Host repacks qw into [notile*16, nkt*128] with rows o*16+r, cols kt*128+c
    # (k-tiles contiguous per out-tile) and sc into [notile*nkt, 128] with rows
    # o*nkt+g.  One 3-dim DMA loads a multi-k-tile sub-slab.
    def rep_src(qw_ap, ot, kt0, nsub, nkt):
        sl = qw_ap[ot * 16:(ot + 1) * 16,
                   kt0 * 128:(kt0 + nsub) * 128]
        return sl.unsqueeze(1).broadcast_to([16, 8, nsub * 128])

    def sc_src(sc_ap, ot, kt0, nsub, nkt):
        sl = sc_ap[ot * nkt + kt0: ot * nkt + kt0 + nsub, :]
        return sl.rearrange("g c -> (g c)").unsqueeze(0).unsqueeze(0) \
                 .broadcast_to([1, 128, nsub * 128])

    with tile.TileContext(nc) as tc:
        ctx_pools = []

        def open_pool(**kw):
            cm = tc.tile_pool(**kw)
            pool = cm.__enter__()
            ctx_pools.append((cm, kw["name"]))
            return pool

        def close_pool(pool_name):
            for i, (cm, nm) in enumerate(ctx_pools):
                if nm == pool_name:
                    cm.__exit__(None, None, None)
                    ctx_pools.pop(i)
                    return

        cp = open_pool(name="const", bufs=1)
        dp = open_pool(name="dram", bufs=1, space="DRAM")
        w4 = open_pool(name="wk4", bufs=2)    # f32t [128,512]
        w6 = open_pool(name="wk6", bufs=4)    # f16t [128,512]
        we = open_pool(name="wke", bufs=2)    # ET [128,512] bf16
        w3 = open_pool(name="wk3", bufs=2)    # rows [1,512] f32
        w2 = open_pool(name="wk2", bufs=2)    # rstdB, onat, rz
        qp = open_pool(name="deq", bufs=2)
        qn = open_pool(name="deq1", bufs=1)   # nibble scratch
        pp = open_pool(name="ps", bufs=2, space="PSUM")
        pro = open_pool(name="pso", bufs=4, space="PSUM")
        prs = open_pool(name="psr", bufs=1, space="PSUM")
        ptr = open_pool(name="pst", bufs=1, space="PSUM")

        # ---- persistent consts in SBUF ----
        shc = cp.tile([128, 1], I32, tag="shc")
        nc.sync.dma_start(out=shc[:], in_=sh_c[:])
        id16 = cp.tile([128, 128], F16, tag="id16")
        nc.sync.dma_start(out=id16[:], in_=id16_c[:])
        onesbf = cp.tile([128, 1], BF16, tag="onesbf")
        nc.sync.dma_start(out=onesbf[:], in_=onesbf_c[:])
        ones16 = cp.tile([128, 1], F16, tag="ones16")
        nc.scalar.activation(ones16[:], onesbf[:], Act.Copy)
        esw16 = cp.tile([128, 63], F16, tag="esw16")
        nc.sync.dma_start(out=esw16[:], in_=esw16_c[:])
        eswbf = cp.tile([128, 63], BF16, tag="eswbf")
        nc.sync.dma_start(out=eswbf[:], in_=eswbf_c[:])
        maskt = cp.tile([128, 896], BF16, tag="maskt")
        nc.sync.dma_start(out=maskt[:], in_=mask_c[:])
        lnw = cp.tile([128, 2 * NKT], F32, tag="lnw")  # ln1 | ln2, [p, kt]
        nc.sync.dma_start(out=lnw[:, 0:NKT],
                          in_=ln1_d[:].rearrange("(kt p) -> p kt", p=128))
        nc.sync.dma_start(out=lnw[:, NKT:2 * NKT],
                          in_=ln2_d[:].rearrange("(kt p) -> p kt", p=128))
        cosr = cp.tile([128, S], F16, tag="cosr")
        nc.sync.dma_start(out=cosr[:], in_=cosT_d[:])
        sinr = cp.tile([128, S], F16, tag="sinr")
        nc.sync.dma_start(out=sinr[:], in_=sinfT_d[:])

        # ---- DRAM scratch ----
        part1_d = [dp.tile([HID, SH], F16, tag=f"part1_{i}", name=f"part1_{i}")
                   for i in range(2)]
        ar1_d = [dp.tile([HID, SH], F16, tag=f"ar1_{i}", name=f"ar1_{i}",
                         addr_space="Shared")
                 for i in range(2)]
        part2a_d = dp.tile([HID // 2, S], F16, tag="part2a")
        part2b_d = dp.tile([HID // 2, S], F16, tag="part2b")
        rs2a_d = dp.tile([OPC // 2, S], F16, tag="rs2a")
        rs2b_d = dp.tile([OPC // 2, S], F16, tag="rs2b")
        wgu_dd = dp.tile([2 * NIT * 128, NKT * 128], BF16, tag="wgu_dd")
        wdn_dd = dp.tile([NKT * 128, NIT * 128], BF16, tag="wdn_dd")

        def f32t():
            return w4.tile([128, 512], F32, tag="f32t", name="f32t")

        def f16t(dtp=F16):
            return w6.tile([128, 512], dtp, tag="f16t", name="f16t")

        # ============ helper: dequant one [nkt*128, 128] slab ============
        # Loads the slab in big DMAs (8 k-tiles each) on sync; nibble-extract
        # and scale-mult run as one wide DVE op pair per 8-k-tile group.
        def dequant_slab(qw_ap, sc_ap, ot, nkt, wdt, w16):
            for kt0 in range(0, nkt, 8):
                nsub = min(8, nkt - kt0)
                qwB = qp.tile([128, 8 * 128], I32, tag="qwB")
                nc.sync.dma_start(out=qwB[:, 0:nsub * 128],
                                  in_=rep_src(qw_ap, ot, kt0, nsub, nkt))
                scB = qp.tile([128, 8 * 128], wdt, tag="scB")
                nc.sync.dma_start(out=scB[:, 0:nsub * 128],
                                  in_=sc_src(sc_ap, ot, kt0, nsub, nkt))
                nib = qn.tile([128, 8 * 128], I32, tag="nib")
                nc.vector.tensor_scalar(
                    out=nib[:, 0:nsub * 128], in0=qwB[:, 0:nsub * 128],
                    scalar1=shc[:], scalar2=15,
                    op0=Alu.logical_shift_right, op1=Alu.bitwise_and)
                nc.vector.tensor_tensor(
                    out=w16[:, kt0 * 128:(kt0 + nsub) * 128],
                    in0=nib[:, 0:nsub * 128],
                    in1=scB[:, 0:nsub * 128], op=Alu.mult)

        def load_zl(nz_ap, ot, ngr, dtp):
            zl = qp.tile([32, 128], dtp, tag="zl")
            nc.sync.dma_start(out=zl[0:ngr, :],
                              in_=nz_ap[0:ngr, ot * 128:(ot + 1) * 128])
            return zl

        # MLP weights pre-dequantized to DRAM (bf16), emitted interleaved
        # with the attention phase so dequant gpsimd/DVE/DMA hides under PE.
        def predeq_unit(u):
            if u < 2 * NIT:
                it = u
                w16 = qp.tile([128, NKT * 128], BF16, tag="w16", name="w16")
                dequant_slab(qw_gu_d[:], sc_gu_d[:], it, NKT, BF16, w16)
                nc.scalar.dma_start(out=wgu_dd[it * 128:(it + 1) * 128, :],
                                    in_=w16[:])
            else:
                ot = u - 2 * NIT
                w16 = qp.tile([128, NKT * 128], BF16, tag="w16", name="w16")
                dequant_slab(qw_dn_d[:], sc_dn_d[:], ot, NIT, BF16, w16)
                nc.scalar.dma_start(out=wdn_dd[ot * 128:(ot + 1) * 128, :],
                                    in_=w16[:, 0:NIT * 128])

        NPRE = 2 * NIT + NKT          # 54 units
        pre_sched = [range(0, 12), range(12, 24), range(24, 36),
                     range(36, 48), range(48, NPRE)]

        # ====================== attention super-phase ======================
        # Per sequence-half: rms1 -> qkv (all heads) -> attention -> o-proj
        # -> fp16 AllReduce of this half's o partials.
        xp = open_pool(name="xph", bufs=1)
        qk2 = open_pool(name="qk2", bufs=1)
        qk1 = open_pool(name="qk1", bufs=1)
        op_ = open_pool(name="oTp", bufs=2)
        opr = open_pool(name="oTr", bufs=1)
        hrp = open_pool(name="hres", bufs=2)

        kTa = qk1.tile([128, NHC * S], F16, tag="kTa")
        Vn = qk1.tile([128, NHC * NST * 132], BF16, tag="Vn")

        for sh in range(2):
            x1T = xp.tile([128, NKT * SH], F16, tag="x1T")
            rsx1 = xp.tile([32, SH], F16, tag="rsx1")
            qTa = qk2.tile([128, NHC * SH], F16, tag="qTa")
            # ---- rms1 for this half's 2 chunks: single h read; stash bf16
            # copy into x1T during the ssq pass, normalize in place after ----
            for ch2 in range(2):
                c0 = sh * SH + ch2 * 512
                c1 = c0 + 512
                ssq_ps = prs.tile([32, 512], F32, tag="rs")
                for st in range(8):
                    hst = hrp.tile([128, 4 * 512], F32, tag="hst", name="hst")
                    nc.gpsimd.dma_start(
                        out=hst[:].rearrange("p (kt s) -> p kt s", s=512),
                        in_=hT_d[st * 512:(st + 1) * 512, c0:c1]
                        .rearrange("(kt p) s -> p kt s", p=128))
                    for k2 in range(4):
                        kt = st * 4 + k2
                        sq = f16t()
                        nc.scalar.activation(
                            sq[:], hst[:, k2 * 512:(k2 + 1) * 512], Act.Square)
                        nc.tensor.matmul(ssq_ps[0:1, :], ones16[:], sq[:],
                                         start=(kt == 0), stop=(kt == NKT - 1))
                        xsl = x1T[:, kt * SH + ch2 * 512: kt * SH + ch2 * 512 + 512]
                        nc.vector.tensor_copy(
                            out=xsl, in_=hst[:, k2 * 512:(k2 + 1) * 512])
                trow = w3.tile([1, 512], F32, tag="rows")
                nc.vector.tensor_scalar(out=trow[:], in0=ssq_ps[0:1, :],
                                        scalar1=1.0 / HID, scalar2=EPS,
                                        op0=Alu.mult, op1=Alu.add)
                rrow = w3.tile([1, 512], F32, tag="rows")
                nc.vector.reciprocal(rrow[:], trow[:])
                srow = w3.tile([1, 512], F16, tag="rowsh")
                nc.scalar.activation(srow[:], rrow[:], Act.Sqrt)
                rstdB = w2.tile([128, 512], F16, tag="rstdB")
                nc.gpsimd.partition_broadcast(rstdB[:], srow[:])
                # in-place normalize + group rowsums
                rsx_ps = prs.tile([32, 512], F32, tag="rs")
                for kt in range(NKT):
                    xsl = x1T[:, kt * SH + ch2 * 512: kt * SH + ch2 * 512 + 512]
                    nc.vector.scalar_tensor_tensor(
                        out=xsl, in0=xsl,
                        scalar=lnw[:, kt:kt + 1],
                        in1=rstdB[:], op0=Alu.mult, op1=Alu.mult)
                    nc.tensor.matmul(rsx_ps[:], esw16[:, 31 - kt:63 - kt],
                                     xsl, start=(kt == 0),
                                     stop=(kt == NKT - 1))
                nc.scalar.activation(rsx1[:, ch2 * 512:(ch2 + 1) * 512],
                                     rsx_ps[:], Act.Copy)
            # ---- qkv for all heads, this half ----
            for h in range(NHC):
                for which in ("q", "k", "v"):
                    ot = {"q": h, "k": NHC + h, "v": 2 * NHC + h}[which]
                    w16 = qp.tile([128, NKT * 128], F16, tag="w16")
                    dequant_slab(qw_qkv_d[:], sc_qkv_d[:], ot, NKT, F16, w16)
                    zl = load_zl(nz_qkv_d[:], ot, 32, F16)
                    for ch2 in range(2):
                        cc = ch2 * 512
                        mm = pp.tile([128, 512], F32, tag="mm")
                        for kt in range(NKT):
                            nc.tensor.matmul(
                                mm[:], w16[:, kt * 128:(kt + 1) * 128],
                                x1T[:, kt * SH + cc: kt * SH + cc + 512],
                                start=(kt == 0), stop=False)
                        nc.tensor.matmul(mm[:], zl[0:32, :],
                                         rsx1[:, cc:cc + 512],
                                         start=False, stop=True)
                        gc0 = sh * SH + cc      # global col
                        if which in ("q", "k"):
                            qsb = f16t()
                            nc.scalar.activation(qsb[:], mm[:], Act.Copy)
                            qsh = f16t()
                            nc.scalar.dma_start(out=qsh[0:64, :],
                                                in_=qsb[64:128, :])
                            nc.scalar.dma_start(out=qsh[64:128, :],
                                                in_=qsb[0:64, :])
                            t1 = f16t()
                            nc.vector.tensor_tensor(
                                out=t1[:], in0=qsb[:],
                                in1=cosr[:, gc0:gc0 + 512], op=Alu.mult)
                            t2 = f16t()
                            nc.vector.tensor_tensor(
                                out=t2[:], in0=qsh[:],
                                in1=sinr[:, gc0:gc0 + 512], op=Alu.mult)
                            if which == "q":
                                dsl = qTa[:, h * SH + cc: h * SH + cc + 512]
                            else:
                                dsl = kTa[:, h * S + gc0: h * S + gc0 + 512]
                            nc.vector.tensor_tensor(out=dsl, in0=t1[:],
                                                    in1=t2[:], op=Alu.add)
                        else:
                            vt = f16t()
                            nc.scalar.activation(vt[:], mm[:], Act.Copy)
                            for st4 in range(4):
                                st = sh * 8 + ch2 * 4 + st4
                                vo = h * NST * 132 + st * 132
                                trp = ptr.tile([128, 128], F16, tag="tr")
                                nc.tensor.transpose(
                                    trp[:], vt[:, st4 * 128:(st4 + 1) * 128],
                                    id16[:])
                                nc.scalar.activation(
                                    Vn[:, vo: vo + 128], trp[:], Act.Copy)
                                nc.vector.memset(Vn[:, vo + 128: vo + 129], 1.0)
            # ---- attention, q-chunks of this half ----
            oTh = op_.tile([128, NHC * SH], F16, tag="oTh")
            rsoh = opr.tile([32, SH], F16, tag="rsoh")
            for qs2 in range(2):
                qs = sh * 2 + qs2
                npairs = 4 * qs + 4
                for h in range(NHC):
                    oap = [pro.tile([128, 264], F32, tag="oa", name="oa")
                           for _ in range(2)]
                    for jj in range(npairs):
                        scp = pp.tile([128, 512], F32, tag="mm")
                        nc.tensor.matmul(
                            scp[:], kTa[:, h * S + jj * 128: h * S + (jj + 1) * 128],
                            qTa[:, h * SH + qs2 * 512: h * SH + qs2 * 512 + 512],
                            start=True, stop=True)
                        ET = we.tile([128, 512], BF16, tag="ET",
                                     name="ET")
                        if jj >= 4 * qs:
                            dd = jj - 4 * qs
                            ms = f32t()
                            nc.vector.scalar_tensor_tensor(
                                out=ms[:], in0=scp[:], scalar=SCALE,
                                in1=maskt[:, 384 - 128 * dd: 896 - 128 * dd],
                                op0=Alu.mult, op1=Alu.add)
                            nc.scalar.activation(ET[:], ms[:], Act.Exp)
                        else:
                            nc.scalar.activation(ET[:], scp[:], Act.Exp,
                                                 scale=SCALE)
                        for qt in range(4):
                            vo = h * NST * 132 + jj * 132
                            qo = (qt % 2) * 132
                            nc.tensor.matmul(
                                oap[qt // 2][:, qo: qo + 129],
                                ET[:, qt * 128:(qt + 1) * 128],
                                Vn[:, vo: vo + 129],
                                start=(jj == 0 and qt % 2 == 0),
                                stop=(jj == npairs - 1),
                                skip_group_check=True)
                    for qt in range(4):
                        oa = oap[qt // 2][:, (qt % 2) * 132:(qt % 2) * 132 + 132]
                        rz = w2.tile([128, 1], F32, tag="rz")
                        nc.vector.reciprocal(rz[:], oa[:, 128:129])
                        onat = w2.tile([128, 128], F16, tag="onat")
                        nc.vector.tensor_scalar(out=onat[:],
                                                in0=oa[:, 0:128],
                                                scalar1=rz[:], scalar2=None,
                                                op0=Alu.mult)
                        trp = ptr.tile([128, 128], F16, tag="tr")
                        nc.tensor.transpose(trp[:], onat[:], id16[:])
                        so = h * SH + (qs2 * 4 + qt) * 128
                        nc.scalar.activation(oTh[:, so: so + 128], trp[:],
                                             Act.Copy)
                for u in pre_sched[sh * 2 + qs2]:
                    predeq_unit(u)
            # ---- rowsums of oTh ----
            for ch2 in range(2):
                cc = ch2 * 512
                rs_ps = prs.tile([32, 512], F32, tag="rs")
                for kt in range(NHC):
                    nc.tensor.matmul(rs_ps[:], esw16[:, 31 - kt:63 - kt],
                                     oTh[:, kt * SH + cc: kt * SH + cc + 512],
                                     start=(kt == 0), stop=(kt == NHC - 1))
                nc.scalar.activation(rsoh[:, cc:cc + 512], rs_ps[:], Act.Copy)
            # ---- o-projection partials for this half -> AllReduce ----
            for ot0 in range(0, NKT, 2):
                w16 = qp.tile([128, NKT * 128], F16, tag="w16")
                qwB = qp.tile([128, 8 * 128], I32, tag="qwB")
                for o2 in range(2):
                    nc.sync.dma_start(
                        out=qwB[:, o2 * 512:(o2 + 1) * 512],
                        in_=rep_src(qw_o_d[:], ot0 + o2, 0, NHC, NHC))
                scB = qp.tile([128, 8 * 128], F16, tag="scB")
                nc.sync.dma_start(out=scB[:, 0:1024],
                                  in_=sc_src(sc_o_d[:], ot0, 0, 8, NHC))
                nib = qn.tile([128, 8 * 128], I32, tag="nib")
                nc.vector.tensor_scalar(
                    out=nib[:, 0:1024], in0=qwB[:, 0:1024],
                    scalar1=shc[:], scalar2=15,
                    op0=Alu.logical_shift_right, op1=Alu.bitwise_and)
                nc.vector.tensor_tensor(
                    out=w16[:, 0:1024], in0=nib[:, 0:1024],
                    in1=scB[:, 0:1024], op=Alu.mult)
                zl = qp.tile([32, 256], F16, tag="zl")
                nc.sync.dma_start(
                    out=zl[0:NHC, :],
                    in_=nz_o_d[0:NHC, ot0 * 128:(ot0 + 2) * 128])
                for o2 in range(2):
                    ot = ot0 + o2
                    for ch2 in range(2):
                        cc = ch2 * 512
                        mm = pp.tile([128, 512], F32, tag="mm")
                        for kt in range(NHC):
                            nc.tensor.matmul(
                                mm[:],
                                w16[:, o2 * 512 + kt * 128: o2 * 512 + (kt + 1) * 128],
                                oTh[:, kt * SH + cc: kt * SH + cc + 512],
                                start=(kt == 0), stop=False)
                        nc.tensor.matmul(mm[:], zl[0:NHC, o2 * 128:(o2 + 1) * 128],
                                         rsoh[0:NHC, cc:cc + 512],
                                         start=False, stop=True)
                        pt = f16t()
                        nc.scalar.activation(pt[:], mm[:], Act.Copy)
                        nc.scalar.dma_start(
                            out=part1_d[sh][ot * 128:(ot + 1) * 128, cc:cc + 512],
                            in_=pt[:])
            nc.gpsimd.collective_compute(
                "AllReduce", Alu.add,
                replica_groups=[list(range(NCORES))],
                ins=[part1_d[sh][:].opt()], outs=[ar1_d[sh][:].opt()])
        close_pool("hres")
        close_pool("oTr")
        close_pool("oTp")
        close_pool("qk1")
        close_pool("qk2")
        close_pool("xph")
        for u in pre_sched[4]:
            predeq_unit(u)

        # ====================== MLP super-phase =============================
        # (weights were pre-dequantized to DRAM, interleaved with attention)
        xp2 = open_pool(name="xph2", bufs=1)
        x2Tc = xp2.tile([128, NKT * 512], BF16, tag="x2Tc")
        yTc = xp2.tile([128, NIT * 512], BF16, tag="yTc")
        h2p = open_pool(name="h2res", bufs=1)
        h2s = open_pool(name="h2strip", bufs=2)

        for ch in range(NCHUNK):
            c0, c1 = ch * 512, (ch + 1) * 512
            arck = ar1_d[ch // 2]
            ac = (ch % 2) * 512
            # ---- fused h2 = h + ar1 (SBUF resident, f16) + rms2 ----
            h2bf = h2p.tile([128, NKT * 512], F16, tag="h2bf")
            rsx2c = w2.tile([32, 512], BF16, tag="rsx2c", name="rsx2c")
            rsyc = w2.tile([32, 512], BF16, tag="rsyc", name="rsyc")
            ssq_ps = prs.tile([32, 512], F32, tag="rs")
            for st in range(8):
                hst = h2s.tile([128, 4 * 512], F32, tag="hst2", name="hst2")
                nc.gpsimd.dma_start(
                    out=hst[:].rearrange("p (kt s) -> p kt s", s=512),
                    in_=hT_d[st * 512:(st + 1) * 512, c0:c1]
                    .rearrange("(kt p) s -> p kt s", p=128))
                ast = h2s.tile([128, 4 * 512], F16, tag="ast", name="ast")
                nc.sync.dma_start(
                    out=ast[:].rearrange("p (kt s) -> p kt s", s=512),
                    in_=arck[st * 512:(st + 1) * 512, ac:ac + 512]
                    .rearrange("(kt p) s -> p kt s", p=128))
                for k2 in range(4):
                    kt = st * 4 + k2
                    sl2 = slice(k2 * 512, (k2 + 1) * 512)
                    sl = slice(kt * 512, (kt + 1) * 512)
                    nc.vector.tensor_tensor(out=h2bf[:, sl], in0=hst[:, sl2],
                                            in1=ast[:, sl2], op=Alu.add)
                    sq = f16t()
                    nc.scalar.activation(sq[:], h2bf[:, sl], Act.Square)
                    nc.tensor.matmul(ssq_ps[0:1, :], ones16[:], sq[:],
                                     start=(kt == 0), stop=(kt == NKT - 1))
            trow = w3.tile([1, 512], F32, tag="rows")
            nc.vector.tensor_scalar(out=trow[:], in0=ssq_ps[0:1, :],
                                    scalar1=1.0 / HID, scalar2=EPS,
                                    op0=Alu.mult, op1=Alu.add)
            rrow = w3.tile([1, 512], F32, tag="rows")
            nc.vector.reciprocal(rrow[:], trow[:])
            srow = w3.tile([1, 512], F16, tag="rowsh")
            nc.scalar.activation(srow[:], rrow[:], Act.Sqrt)
            rstdB = w2.tile([128, 512], F16, tag="rstdB")
            nc.gpsimd.partition_broadcast(rstdB[:], srow[:])
            rsx_ps = prs.tile([32, 512], F32, tag="rs")
            for kt in range(NKT):
                xsl = x2Tc[:, kt * 512:(kt + 1) * 512]
                nc.vector.scalar_tensor_tensor(
                    out=xsl, in0=h2bf[:, kt * 512:(kt + 1) * 512],
                    scalar=lnw[:, NKT + kt:NKT + kt + 1],
                    in1=rstdB[:], op0=Alu.mult, op1=Alu.mult)
                nc.tensor.matmul(rsx_ps[:], eswbf[:, 31 - kt:63 - kt],
                                 xsl, start=(kt == 0), stop=(kt == NKT - 1))
            nc.scalar.activation(rsx2c[:], rsx_ps[:], Act.Copy)
            # gate/up -> yTc
            for it in range(NIT):
                wg = qp.tile([128, NKT * 128], BF16, tag="w16")
                nc.scalar.dma_start(out=wg[:],
                                    in_=wgu_dd[it * 128:(it + 1) * 128, :])
                zlg = load_zl(nz_gu_d[:], it, 32, BF16)
                wu = qp.tile([128, NKT * 128], BF16, tag="w16")
                nc.scalar.dma_start(
                    out=wu[:],
                    in_=wgu_dd[(NIT + it) * 128:(NIT + it + 1) * 128, :])
                zlu = load_zl(nz_gu_d[:], NIT + it, 32, BF16)
                gp = pp.tile([128, 512], F32, tag="mm")
                for kt in range(NKT):
                    nc.tensor.matmul(
                        gp[:], wg[:, kt * 128:(kt + 1) * 128],
                        x2Tc[:, kt * 512:(kt + 1) * 512],
                        start=(kt == 0), stop=False)
                nc.tensor.matmul(gp[:], zlg[0:32, :], rsx2c[:],
                                 start=False, stop=True)
                up = pp.tile([128, 512], F32, tag="mm")
                for kt in range(NKT):
                    nc.tensor.matmul(
                        up[:], wu[:, kt * 128:(kt + 1) * 128],
                        x2Tc[:, kt * 512:(kt + 1) * 512],
                        start=(kt == 0), stop=False)
                nc.tensor.matmul(up[:], zlu[0:32, :], rsx2c[:],
                                 start=False, stop=True)
                sg = f32t()
                nc.scalar.activation(sg[:], gp[:], Act.Silu)
                nc.vector.tensor_tensor(
                    out=yTc[:, it * 512:(it + 1) * 512],
                    in0=sg[:], in1=up[:], op=Alu.mult)
            # rowsums of yTc
            rs_ps = prs.tile([32, 512], F32, tag="rs")
            for kt in range(NIT):
                nc.tensor.matmul(rs_ps[:], eswbf[:, 31 - kt:63 - kt],
                                 yTc[:, kt * 512:(kt + 1) * 512],
                                 start=(kt == 0), stop=(kt == NIT - 1))
            nc.scalar.activation(rsyc[:], rs_ps[:], Act.Copy)
            # down partials + h2/8 -> part2{a,b}_d
            for ot in range(NKT):
                w16 = qp.tile([128, NKT * 128], BF16, tag="w16")
                nc.scalar.dma_start(out=w16[:, 0:NIT * 128],
                                    in_=wdn_dd[ot * 128:(ot + 1) * 128, :])
                zl = load_zl(nz_dn_d[:], ot, NIT, BF16)
                mm = pp.tile([128, 512], F32, tag="mm")
                for kt in range(NIT):
                    nc.tensor.matmul(
                        mm[:], w16[:, kt * 128:(kt + 1) * 128],
                        yTc[:, kt * 512:(kt + 1) * 512],
                        start=(kt == 0), stop=False)
                nc.tensor.matmul(mm[:], zl[0:NIT, :], rsyc[0:NIT, :],
                                 start=False, stop=True)
                pt = f16t()
                nc.vector.scalar_tensor_tensor(
                    out=pt[:], in0=h2bf[:, ot * 512:(ot + 1) * 512],
                    scalar=1.0 / NCORES, in1=mm[:],
                    op0=Alu.mult, op1=Alu.add)
                dst_d = part2a_d if ot < HKT else part2b_d
                ro = (ot % HKT) * 128
                nc.scalar.dma_start(out=dst_d[ro:ro + 128, c0:c1], in_=pt[:])
        close_pool("h2strip")
        close_pool("h2res")
        close_pool("xph2")

        # =========== exchange 2: 2x half ReduceScatter -> out ===========
        nc.gpsimd.collective_compute(
            "ReduceScatter", Alu.add,
            replica_groups=[list(range(NCORES))],
            ins=[part2a_d[:].opt()], outs=[rs2a_d[:].opt()])
        nc.gpsimd.collective_compute(
            "ReduceScatter", Alu.add,
            replica_groups=[list(range(NCORES))],
            ins=[part2b_d[:].opt()], outs=[rs2b_d[:].opt()])
        nc.sync.dma_start(out=outA_d[:], in_=rs2a_d[:])
        nc.sync.dma_start(out=outB_d[:], in_=rs2b_d[:])

        for cm, nm in reversed(ctx_pools):
            cm.__exit__(None, None, None)
        ctx_pools.clear()

    nc.compile()
    return nc


def _host_prep(inputs):
    """Build the 8 per-core input maps from full inputs."""
    import ml_dtypes
    bf16 = ml_dtypes.bfloat16
    f16 = np.float16

    def unpack_z1(qz):
        sh = (np.arange(8, dtype=np.uint32) * 4)
        z = ((qz[:, :, None].view(np.uint32) >> sh[None, None, :]) & 15)
        return z.reshape(qz.shape[0], -1).astype(np.float32) + 1.0

    h = np.asarray(inputs["hidden_states"], np.float32)[0]     # [S, HID]
    hT = np.ascontiguousarray(h.T)                             # [HID, S]
    sin = np.asarray(inputs["sin"], np.float32)                # [S, HD]
    cos = np.asarray(inputs["cos"], np.float32)
    cosT = np.ascontiguousarray(cos.T).astype(f16)
    sinf = sin.T.copy()
    sinf[0:64, :] *= -1.0                                      # rot-half sign fold
    sinfT = np.ascontiguousarray(sinf).astype(f16)

    qkv_qw, qkv_sc, qkv_nz = [], [], []
    for nm in ("q", "k", "v"):
        qw = np.asarray(inputs["qw_" + nm])
        sc = np.asarray(inputs["sc_" + nm], np.float32)
        z1 = unpack_z1(np.asarray(inputs["qz_" + nm]))
        qkv_qw.append(qw); qkv_sc.append(sc); qkv_nz.append(-(z1 * sc))

    qw_o = np.asarray(inputs["qw_o"])
    sc_o = np.asarray(inputs["sc_o"], np.float32)
    nz_o = -(unpack_z1(np.asarray(inputs["qz_o"])) * sc_o)

    def pad_cols(a, w):
        out = np.zeros((a.shape[0], w), a.dtype)
        out[:, :a.shape[1]] = a
        return out

    qw_g = pad_cols(np.asarray(inputs["qw_gate"]), IPAD)
    qw_u = pad_cols(np.asarray(inputs["qw_up"]), IPAD)
    sc_g = pad_cols(np.asarray(inputs["sc_gate"], np.float32), IPAD)
    sc_u = pad_cols(np.asarray(inputs["sc_up"], np.float32), IPAD)
    nz_g = pad_cols(-(unpack_z1(np.asarray(inputs["qz_gate"]))
                      * np.asarray(inputs["sc_gate"], np.float32)), IPAD)
    nz_u = pad_cols(-(unpack_z1(np.asarray(inputs["qz_up"]))
                      * np.asarray(inputs["sc_up"], np.float32)), IPAD)

    qw_dn = np.zeros((IPAD // 8, HID), np.int32)
    qw_dn[:INTER // 8] = np.asarray(inputs["qw_down"])
    sc_dn = np.zeros((IPAD // GS, HID), np.float32)
    sc_dn[:INTER // GS] = np.asarray(inputs["sc_down"], np.float32)
    nz_dn = np.zeros((IPAD // GS, HID), np.float32)
    nz_dn[:INTER // GS] = -(unpack_z1(np.asarray(inputs["qz_down"]))
                            * np.asarray(inputs["sc_down"], np.float32))

    ln1 = np.asarray(inputs["ln1_w"], np.float32)
    ln2 = np.asarray(inputs["ln2_w"], np.float32)

    def repack_qw(qw):
        # [nkt*16, notile*128] -> [notile*16, nkt*128], rows o*16+r,
        # k-tiles contiguous per out-tile
        nkt = qw.shape[0] // 16
        notile = qw.shape[1] // 128
        return np.ascontiguousarray(
            qw.reshape(nkt, 16, notile, 128).transpose(2, 1, 0, 3)
            .reshape(notile * 16, nkt * 128))

    def repack_sc(sc):
        # [G, notile*128] -> [notile*G, 128], rows o*G+g
        G = sc.shape[0]
        notile = sc.shape[1] // 128
        return np.ascontiguousarray(
            sc.reshape(G, notile, 128).transpose(1, 0, 2)
            .reshape(notile * G, 128))

    maps = []
    for c in range(NCORES):
        cs = slice(c * OPC, (c + 1) * OPC)
        isl = slice(c * IPC, (c + 1) * IPC)
        m = {
            "hT": hT, "cosT": cosT, "sinfT": sinfT, "ln1": ln1, "ln2": ln2,
            "qw_qkv": repack_qw(
                np.concatenate([qkv_qw[i][:, cs] for i in range(3)], axis=1)),
            "sc_qkv": repack_sc(
                np.concatenate([qkv_sc[i][:, cs] for i in range(3)],
                               axis=1).astype(f16)),
            "nz_qkv": np.ascontiguousarray(
                np.concatenate([qkv_nz[i][:, cs] for i in range(3)],
                               axis=1)).astype(f16),
            "qw_o": repack_qw(qw_o[c * OPC // 8:(c + 1) * OPC // 8]),
            "sc_o": repack_sc(
                sc_o[c * OPC // GS:(c + 1) * OPC // GS].astype(f16)),
            "nz_o": np.ascontiguousarray(
                nz_o[c * OPC // GS:(c + 1) * OPC // GS]).astype(f16),
            "qw_gu": repack_qw(
                np.concatenate([qw_g[:, isl], qw_u[:, isl]], axis=1)),
            "sc_gu": repack_sc(
                np.concatenate([sc_g[:, isl], sc_u[:, isl]],
                               axis=1).astype(bf16)),
            "nz_gu": np.ascontiguousarray(
                np.concatenate([nz_g[:, isl], nz_u[:, isl]],
                               axis=1)).astype(bf16),
            "qw_dn": repack_qw(qw_dn[c * IPC // 8:(c + 1) * IPC // 8]),
            "sc_dn": repack_sc(
                sc_dn[c * NIT:(c + 1) * NIT].astype(bf16)),
            "nz_dn": np.ascontiguousarray(
                nz_dn[c * NIT:(c + 1) * NIT]).astype(bf16),
        }
        maps.append(m)
    return maps


def run(inputs, trace=False):
    from concourse.bass_utils import run_bass_kernel_spmd
    if "rel" not in _BUILD_CACHE:
        _BUILD_CACHE["rel"] = _build()
    nc = _BUILD_CACHE["rel"]
    maps = _host_prep(inputs)
    res = run_bass_kernel_spmd(nc, maps, core_ids=list(range(NCORES)),
                               trace=trace)
    HO = OPC // 2
    outT = np.empty((HID, S), np.float32)
    for c in range(NCORES):
        outT[c * HO:(c + 1) * HO] = res.results[c]["outA"]
        outT[HID // 2 + c * HO: HID // 2 + (c + 1) * HO] = res.results[c]["outB"]
    out = np.ascontiguousarray(outT.T)[None]
    return out, res


def kernel(**inputs):
    out, _ = run(inputs)
    return out


# revision 32
# speedup vs baseline: 1.0438x; 1.0438x over previous
"""ExLlama transformer layer (GPTQ int4) on 8 TRN2 NeuronCores, tensor-parallel.

Self-contained: hardcodes shapes from the problem spec.
  B=1, S=2048, HID=4096, INTER=11008, HEADS=32, HD=128, GS=128.

Sharding (SPMD, identical program per core, per-core data slices):
  - q/k/v column-sharded (4 heads per core), attention head-parallel
  - o row-sharded -> partial sums -> fp16 AllReduce per seq-half -> h2
  - gate/up column-sharded over padded INTER (8*1408=11264, zero-padded)
  - down row-sharded -> partials (+h2/8) -> fp16 ReduceScatter per
    feature-half; host reassembles the permuted feature order.

The attention super-phase runs per sequence-half (x1 only half-resident in
SBUF), so the first half's AllReduce overlaps the second half's compute.
h2 = h + ar is built SBUF-resident (bf16) fused into the MLP rms pass; the
residual rides through the down partials as +h2/8.
"""
import sys

sys.path.insert(0, "/opt/trn_rl_repo")

import numpy as np

S = 2048
HID = 4096
HD = 128
GS = 128
INTER = 11008
NCORES = 8
IPC = 1408                      # padded inter features per core
IPAD = IPC * NCORES             # 11264
NKT = HID // 128                # 32 k-tiles over HID
NIT = IPC // 128                # 11 k-tiles over per-core inter
OPC = HID // NCORES             # 512 out features per core (qkv), 4 heads
NHC = OPC // HD                 # 4 heads per core
CHUNK = 512
NCHUNK = S // CHUNK             # 4
NST = S // 128                  # 16 s-tiles
HKT = NKT // 2                  # 16 k-tiles per feature half
SH = S // 2                     # 1024, seq half
SCALE = 1.0 / float(np.sqrt(HD))
EPS = 1e-6
NEG = -30000.0

_BUILD_CACHE = {}


def _build():
    import concourse.bacc as bacc
    import concourse.mybir as mybir
    import concourse.tile as tile
    import ml_dtypes

    dt = mybir.dt
    F32, F16, BF16, I32 = dt.float32, dt.float16, dt.bfloat16, dt.int32
    Alu = mybir.AluOpType
    Act = mybir.ActivationFunctionType

    nc = bacc.Bacc("TRN2", target_bir_lowering=False, num_devices=NCORES)

    # ---------------- external I/O ----------------
    hT_d = nc.dram_tensor("hT", [HID, S], F32, kind="ExternalInput")
    cosT_d = nc.dram_tensor("cosT", [HD, S], F16, kind="ExternalInput")
    sinfT_d = nc.dram_tensor("sinfT", [HD, S], F16, kind="ExternalInput")
    ln1_d = nc.dram_tensor("ln1", [HID], F32, kind="ExternalInput")
    ln2_d = nc.dram_tensor("ln2", [HID], F32, kind="ExternalInput")
    qw_qkv_d = nc.dram_tensor("qw_qkv", [12 * 16, NKT * 128], I32, kind="ExternalInput")
    sc_qkv_d = nc.dram_tensor("sc_qkv", [12 * NKT, 128], F16, kind="ExternalInput")
    nz_qkv_d = nc.dram_tensor("nz_qkv", [32, 3 * OPC], F16, kind="ExternalInput")
    qw_o_d = nc.dram_tensor("qw_o", [NKT * 16, NHC * 128], I32, kind="ExternalInput")
    sc_o_d = nc.dram_tensor("sc_o", [NKT * NHC, 128], F16, kind="ExternalInput")
    nz_o_d = nc.dram_tensor("nz_o", [OPC // GS, HID], F16, kind="ExternalInput")
    qw_gu_d = nc.dram_tensor("qw_gu", [2 * NIT * 16, NKT * 128], I32, kind="ExternalInput")
    sc_gu_d = nc.dram_tensor("sc_gu", [2 * NIT * NKT, 128], BF16, kind="ExternalInput")
    nz_gu_d = nc.dram_tensor("nz_gu", [32, 2 * IPC], BF16, kind="ExternalInput")
    qw_dn_d = nc.dram_tensor("qw_dn", [NKT * 16, NIT * 128], I32, kind="ExternalInput")
    sc_dn_d = nc.dram_tensor("sc_dn", [NKT * NIT, 128], BF16, kind="ExternalInput")
    nz_dn_d = nc.dram_tensor("nz_dn", [NIT, HID], BF16, kind="ExternalInput")

    outA_d = nc.dram_tensor("outA", [OPC // 2, S], F16, kind="ExternalOutput")
    outB_d = nc.dram_tensor("outB", [OPC // 2, S], F16, kind="ExternalOutput")

    # ---------------- inline constants ----------------
    p = np.arange(128)
    sh_c = nc.inline_tensor(((p % 8) * 4).astype(np.int32)[:, None], name="shc")
    id16_c = nc.inline_tensor(np.eye(128, dtype=np.float16), name="id16")
    onesbf_c = nc.inline_tensor(np.ones((128, 1), ml_dtypes.bfloat16),
                                name="onesbf")
    # wide selector: eslw[:, j] = 1 iff j == 31; slice [31-t : 63-t] gives a
    # [128, 32] tile whose column t is all-ones (rowsum-per-group lhsT).
    eslw = np.zeros((128, 63), np.float16)
    eslw[:, 31] = 1.0
    esw16_c = nc.inline_tensor(eslw, name="esw16")
    eswbf_c = nc.inline_tensor(eslw.astype(ml_dtypes.bfloat16), name="eswbf")
    # wide causal mask, S^T layout: mkw[k, j] with qq = j-384: 0 if qq >= k
    # else NEG.  Slice [384-128*dd : 896-128*dd] = mask for diag offset dd.
    j = np.arange(896)
    mkw = np.where((j[None, :] - 384) >= p[:, None], 0.0, NEG)
    mask_c = nc.inline_tensor(mkw.astype(ml_dtypes.bfloat16), name="maskc")

    # Host repacks qw into [notile*16, nkt*128] with rows o*16+r, cols kt*128+c
    # (k-tiles contiguous per out-tile) and sc into [notile*nkt, 128] with rows
    # o*nkt+g.  One 3-dim DMA loads a multi-k-tile sub-slab.
    def rep_src(qw_ap, ot, kt0, nsub, nkt):
        sl = qw_ap[ot * 16:(ot + 1) * 16,
                   kt0 * 128:(kt0 + nsub) * 128]
        return sl.unsqueeze(1).broadcast_to([16, 8, nsub * 128])

    def sc_src(sc_ap, ot, kt0, nsub, nkt):
        sl = sc_ap[ot * nkt + kt0: ot * nkt + kt0 + nsub, :]
        return sl.rearrange("g c -> (g c)").unsqueeze(0).unsqueeze(0) \
                 .broadcast_to([1, 128, nsub * 128])

    with tile.TileContext(nc) as tc:
        ctx_pools = []

        def open_pool(**kw):
            cm = tc.tile_pool(**kw)
            pool = cm.__enter__()
            ctx_pools.append((cm, kw["name"]))
            return pool

        def close_pool(pool_name):
            for i, (cm, nm) in enumerate(ctx_pools):
                if nm == pool_name:
                    cm.__exit__(None, None, None)
                    ctx_pools.pop(i)
                    return

        cp = open_pool(name="const", bufs=1)
        dp = open_pool(name="dram", bufs=1, space="DRAM")
        w4 = open_pool(name="wk4", bufs=2)    # f32t [128,512]
        w6 = open_pool(name="wk6", bufs=4)    # f16t [128,512]
        we = open_pool(name="wke", bufs=3)    # ET [128,512] bf16
        w3 = open_pool(name="wk3", bufs=2)    # rows [1,512] f32
        w2 = open_pool(name="wk2", bufs=2)    # rstdB, onat, rz
        qp = open_pool(name="deq", bufs=2)
        qn = open_pool(name="deq1", bufs=1)   # nibble scratch
        pp = open_pool(name="ps", bufs=2, space="PSUM")
        pro = open_pool(name="pso", bufs=4, space="PSUM")
        prs = open_pool(name="psr", bufs=1, space="PSUM")
        ptr = open_pool(name="pst", bufs=1, space="PSUM")

        # ---- persistent consts in SBUF ----
        shc = cp.tile([128, 1], I32, tag="shc")
        nc.sync.dma_start(out=shc[:], in_=sh_c[:])
        id16 = cp.tile([128, 128], F16, tag="id16")
        nc.sync.dma_start(out=id16[:], in_=id16_c[:])
        onesbf = cp.tile([128, 1], BF16, tag="onesbf")
        nc.sync.dma_start(out=onesbf[:], in_=onesbf_c[:])
        ones16 = cp.tile([128, 1], F16, tag="ones16")
        nc.scalar.activation(ones16[:], onesbf[:], Act.Copy)
        esw16 = cp.tile([128, 63], F16, tag="esw16")
        nc.sync.dma_start(out=esw16[:], in_=esw16_c[:])
        eswbf = cp.tile([128, 63], BF16, tag="eswbf")
        nc.sync.dma_start(out=eswbf[:], in_=eswbf_c[:])
        maskt = cp.tile([128, 896], BF16, tag="maskt")
        nc.sync.dma_start(out=maskt[:], in_=mask_c[:])
        lnw = cp.tile([128, 2 * NKT], F32, tag="lnw")  # ln1 | ln2, [p, kt]
        nc.sync.dma_start(out=lnw[:, 0:NKT],
                          in_=ln1_d[:].rearrange("(kt p) -> p kt", p=128))
        nc.sync.dma_start(out=lnw[:, NKT:2 * NKT],
                          in_=ln2_d[:].rearrange("(kt p) -> p kt", p=128))
        cosr = cp.tile([128, S], F16, tag="cosr")
        nc.sync.dma_start(out=cosr[:], in_=cosT_d[:])
        sinr = cp.tile([128, S], F16, tag="sinr")
        nc.sync.dma_start(out=sinr[:], in_=sinfT_d[:])

        # ---- DRAM scratch ----
        part1_d = [dp.tile([HID, SH], F16, tag=f"part1_{i}", name=f"part1_{i}")
                   for i in range(2)]
        ar1_d = [dp.tile([HID, SH], F16, tag=f"ar1_{i}", name=f"ar1_{i}",
                         addr_space="Shared")
                 for i in range(2)]
        part2a_d = dp.tile([HID // 2, S], F16, tag="part2a")
        part2b_d = dp.tile([HID // 2, S], F16, tag="part2b")
        rs2a_d = dp.tile([OPC // 2, S], F16, tag="rs2a")
        rs2b_d = dp.tile([OPC // 2, S], F16, tag="rs2b")
        wgu_dd = dp.tile([2 * NIT * 128, NKT * 128], BF16, tag="wgu_dd")
        wdn_dd = dp.tile([NKT * 128, NIT * 128], BF16, tag="wdn_dd")

        def f32t():
            return w4.tile([128, 512], F32, tag="f32t", name="f32t")

        def f16t(dtp=F16):
            return w6.tile([128, 512], dtp, tag="f16t", name="f16t")

        # ============ helper: dequant one [nkt*128, 128] slab ============
        # Loads the slab in big DMAs (8 k-tiles each) on sync; nibble-extract
        # and scale-mult run as one wide DVE op pair per 8-k-tile group.
        def dequant_slab(qw_ap, sc_ap, ot, nkt, wdt, w16):
            for kt0 in range(0, nkt, 8):
                nsub = min(8, nkt - kt0)
                qwB = qp.tile([128, 8 * 128], I32, tag="qwB")
                nc.sync.dma_start(out=qwB[:, 0:nsub * 128],
                                  in_=rep_src(qw_ap, ot, kt0, nsub, nkt))
                scB = qp.tile([128, 8 * 128], wdt, tag="scB")
                nc.sync.dma_start(out=scB[:, 0:nsub * 128],
                                  in_=sc_src(sc_ap, ot, kt0, nsub, nkt))
                nib = qn.tile([128, 8 * 128], I32, tag="nib")
                nc.vector.tensor_scalar(
                    out=nib[:, 0:nsub * 128], in0=qwB[:, 0:nsub * 128],
                    scalar1=shc[:], scalar2=15,
                    op0=Alu.logical_shift_right, op1=Alu.bitwise_and)
                nc.vector.tensor_tensor(
                    out=w16[:, kt0 * 128:(kt0 + nsub) * 128],
                    in0=nib[:, 0:nsub * 128],
                    in1=scB[:, 0:nsub * 128], op=Alu.mult)

        def load_zl(nz_ap, ot, ngr, dtp):
            zl = qp.tile([32, 128], dtp, tag="zl")
            nc.sync.dma_start(out=zl[0:ngr, :],
                              in_=nz_ap[0:ngr, ot * 128:(ot + 1) * 128])
            return zl

        # MLP weights pre-dequantized to DRAM (bf16), emitted interleaved
        # with the attention phase so dequant gpsimd/DVE/DMA hides under PE.
        def predeq_unit(u):
            if u < 2 * NIT:
                it = u
                w16 = qp.tile([128, NKT * 128], BF16, tag="w16", name="w16")
                dequant_slab(qw_gu_d[:], sc_gu_d[:], it, NKT, BF16, w16)
                nc.scalar.dma_start(out=wgu_dd[it * 128:(it + 1) * 128, :],
                                    in_=w16[:])
            else:
                ot = u - 2 * NIT
                w16 = qp.tile([128, NKT * 128], BF16, tag="w16", name="w16")
                dequant_slab(qw_dn_d[:], sc_dn_d[:], ot, NIT, BF16, w16)
                nc.scalar.dma_start(out=wdn_dd[ot * 128:(ot + 1) * 128, :],
                                    in_=w16[:, 0:NIT * 128])

        NPRE = 2 * NIT + NKT          # 54 units
        pre_sched = [range(0, 12), range(12, 24), range(24, 36),
                     range(36, 48), range(48, NPRE)]

        # ====================== attention super-phase ======================
        # Per sequence-half: rms1 -> qkv (all heads) -> attention -> o-proj
        # -> fp16 AllReduce of this half's o partials.
        xp = open_pool(name="xph", bufs=1)
        qk2 = open_pool(name="qk2", bufs=1)
        qk1 = open_pool(name="qk1", bufs=1)
        op_ = open_pool(name="oTp", bufs=2)
        opr = open_pool(name="oTr", bufs=1)
        hrp = open_pool(name="hres", bufs=2)

        kTa = qk1.tile([128, NHC * S], F16, tag="kTa")
        Vn = qk1.tile([128, NHC * NST * 132], BF16, tag="Vn")

        def rms1_half(sh):
            x1T = xp.tile([128, NKT * SH], F16, tag="x1T", name="x1T")
            rsx1 = xp.tile([32, SH], F16, tag="rsx1", name="rsx1")
            # single h read; stash f16 copy into x1T during the ssq pass,
            # normalize in place after
            for ch2 in range(2):
                c0 = sh * SH + ch2 * 512
                c1 = c0 + 512
                ssq_ps = prs.tile([32, 512], F32, tag="rs")
                for st in range(8):
                    hst = hrp.tile([128, 4 * 512], F32, tag="hst", name="hst")
                    nc.gpsimd.dma_start(
                        out=hst[:].rearrange("p (kt s) -> p kt s", s=512),
                        in_=hT_d[st * 512:(st + 1) * 512, c0:c1]
                        .rearrange("(kt p) s -> p kt s", p=128))
                    for k2 in range(4):
                        kt = st * 4 + k2
                        sq = f16t()
                        nc.scalar.activation(
                            sq[:], hst[:, k2 * 512:(k2 + 1) * 512], Act.Square)
                        nc.tensor.matmul(ssq_ps[0:1, :], ones16[:], sq[:],
                                         start=(kt == 0), stop=(kt == NKT - 1))
                        xsl = x1T[:, kt * SH + ch2 * 512: kt * SH + ch2 * 512 + 512]
                        nc.vector.tensor_copy(
                            out=xsl, in_=hst[:, k2 * 512:(k2 + 1) * 512])
                trow = w3.tile([1, 512], F32, tag="rows")
                nc.vector.tensor_scalar(out=trow[:], in0=ssq_ps[0:1, :],
                                        scalar1=1.0 / HID, scalar2=EPS,
                                        op0=Alu.mult, op1=Alu.add)
                rrow = w3.tile([1, 512], F32, tag="rows")
                nc.vector.reciprocal(rrow[:], trow[:])
                srow = w3.tile([1, 512], F16, tag="rowsh")
                nc.scalar.activation(srow[:], rrow[:], Act.Sqrt)
                rstdB = w2.tile([128, 512], F16, tag="rstdB")
                nc.gpsimd.partition_broadcast(rstdB[:], srow[:])
                # in-place normalize + group rowsums
                rsx_ps = prs.tile([32, 512], F32, tag="rs")
                for kt in range(NKT):
                    xsl = x1T[:, kt * SH + ch2 * 512: kt * SH + ch2 * 512 + 512]
                    nc.vector.scalar_tensor_tensor(
                        out=xsl, in0=xsl,
                        scalar=lnw[:, kt:kt + 1],
                        in1=rstdB[:], op0=Alu.mult, op1=Alu.mult)
                    nc.tensor.matmul(rsx_ps[:], esw16[:, 31 - kt:63 - kt],
                                     xsl, start=(kt == 0),
                                     stop=(kt == NKT - 1))
                nc.scalar.activation(rsx1[:, ch2 * 512:(ch2 + 1) * 512],
                                     rsx_ps[:], Act.Copy)
            return x1T, rsx1

        xr = rms1_half(0)
        for sh in range(2):
            x1T, rsx1 = xr
            qTa = qk2.tile([128, NHC * SH], F16, tag="qTa")
            # ---- qkv for all heads, this half ----
            for h in range(NHC):
                for which in ("q", "k", "v"):
                    ot = {"q": h, "k": NHC + h, "v": 2 * NHC + h}[which]
                    w16 = qp.tile([128, NKT * 128], F16, tag="w16")
                    dequant_slab(qw_qkv_d[:], sc_qkv_d[:], ot, NKT, F16, w16)
                    zl = load_zl(nz_qkv_d[:], ot, 32, F16)
                    for ch2 in range(2):
                        cc = ch2 * 512
                        mm = pp.tile([128, 512], F32, tag="mm")
                        for kt in range(NKT):
                            nc.tensor.matmul(
                                mm[:], w16[:, kt * 128:(kt + 1) * 128],
                                x1T[:, kt * SH + cc: kt * SH + cc + 512],
                                start=(kt == 0), stop=False)
                        nc.tensor.matmul(mm[:], zl[0:32, :],
                                         rsx1[:, cc:cc + 512],
                                         start=False, stop=True)
                        gc0 = sh * SH + cc      # global col
                        if which in ("q", "k"):
                            qsb = f16t()
                            nc.scalar.activation(qsb[:], mm[:], Act.Copy)
                            qsh = f16t()
                            nc.scalar.dma_start(out=qsh[0:64, :],
                                                in_=qsb[64:128, :])
                            nc.scalar.dma_start(out=qsh[64:128, :],
                                                in_=qsb[0:64, :])
                            t1 = f16t()
                            nc.vector.tensor_tensor(
                                out=t1[:], in0=qsb[:],
                                in1=cosr[:, gc0:gc0 + 512], op=Alu.mult)
                            t2 = f16t()
                            nc.vector.tensor_tensor(
                                out=t2[:], in0=qsh[:],
                                in1=sinr[:, gc0:gc0 + 512], op=Alu.mult)
                            if which == "q":
                                dsl = qTa[:, h * SH + cc: h * SH + cc + 512]
                            else:
                                dsl = kTa[:, h * S + gc0: h * S + gc0 + 512]
                            nc.vector.tensor_tensor(out=dsl, in0=t1[:],
                                                    in1=t2[:], op=Alu.add)
                        else:
                            vt = f16t()
                            nc.scalar.activation(vt[:], mm[:], Act.Copy)
                            for st4 in range(4):
                                st = sh * 8 + ch2 * 4 + st4
                                vo = h * NST * 132 + st * 132
                                trp = ptr.tile([128, 128], F16, tag="tr")
                                nc.tensor.transpose(
                                    trp[:], vt[:, st4 * 128:(st4 + 1) * 128],
                                    id16[:])
                                nc.scalar.activation(
                                    Vn[:, vo: vo + 128], trp[:], Act.Copy)
                                nc.vector.memset(Vn[:, vo + 128: vo + 129], 1.0)
            if sh == 0:
                xr = rms1_half(1)
            # ---- attention, q-chunks of this half ----
            oTh = op_.tile([128, NHC * SH], F16, tag="oTh")
            rsoh = opr.tile([32, SH], F16, tag="rsoh")
            for qs2 in range(2):
                qs = sh * 2 + qs2
                npairs = 4 * qs + 4
                for h in range(NHC):
                    oap = [pro.tile([128, 264], F32, tag="oa", name="oa")
                           for _ in range(2)]
                    for jj in range(npairs):
                        scp = pp.tile([128, 512], F32, tag="mm")
                        nc.tensor.matmul(
                            scp[:], kTa[:, h * S + jj * 128: h * S + (jj + 1) * 128],
                            qTa[:, h * SH + qs2 * 512: h * SH + qs2 * 512 + 512],
                            start=True, stop=True)
                        ET = we.tile([128, 512], BF16, tag="ET",
                                     name="ET")
                        if jj >= 4 * qs:
                            dd = jj - 4 * qs
                            ms = f32t()
                            nc.vector.scalar_tensor_tensor(
                                out=ms[:], in0=scp[:], scalar=SCALE,
                                in1=maskt[:, 384 - 128 * dd: 896 - 128 * dd],
                                op0=Alu.mult, op1=Alu.add)
                            nc.scalar.activation(ET[:], ms[:], Act.Exp)
                        else:
                            nc.scalar.activation(ET[:], scp[:], Act.Exp,
                                                 scale=SCALE)
                        for qt in range(4):
                            vo = h * NST * 132 + jj * 132
                            qo = (qt % 2) * 132
                            nc.tensor.matmul(
                                oap[qt // 2][:, qo: qo + 129],
                                ET[:, qt * 128:(qt + 1) * 128],
                                Vn[:, vo: vo + 129],
                                start=(jj == 0 and qt % 2 == 0),
                                stop=(jj == npairs - 1),
                                skip_group_check=True)
                    for qt in range(4):
                        oa = oap[qt // 2][:, (qt % 2) * 132:(qt % 2) * 132 + 132]
                        rz = w2.tile([128, 1], F32, tag="rz")
                        nc.vector.reciprocal(rz[:], oa[:, 128:129])
                        onat = w2.tile([128, 128], F16, tag="onat")
                        nc.vector.tensor_scalar(out=onat[:],
                                                in0=oa[:, 0:128],
                                                scalar1=rz[:], scalar2=None,
                                                op0=Alu.mult)
                        trp = ptr.tile([128, 128], F16, tag="tr")
                        nc.tensor.transpose(trp[:], onat[:], id16[:])
                        so = h * SH + (qs2 * 4 + qt) * 128
                        nc.scalar.activation(oTh[:, so: so + 128], trp[:],
                                             Act.Copy)
                for u in pre_sched[sh * 2 + qs2]:
                    predeq_unit(u)
            # ---- rowsums of oTh ----
            for ch2 in range(2):
                cc = ch2 * 512
                rs_ps = prs.tile([32, 512], F32, tag="rs")
                for kt in range(NHC):
                    nc.tensor.matmul(rs_ps[:], esw16[:, 31 - kt:63 - kt],
                                     oTh[:, kt * SH + cc: kt * SH + cc + 512],
                                     start=(kt == 0), stop=(kt == NHC - 1))
                nc.scalar.activation(rsoh[:, cc:cc + 512], rs_ps[:], Act.Copy)
            # ---- o-projection partials for this half -> AllReduce ----
            for ot0 in range(0, NKT, 2):
                w16 = qp.tile([128, NKT * 128], F16, tag="w16")
                qwB = qp.tile([128, 8 * 128], I32, tag="qwB")
                for o2 in range(2):
                    nc.sync.dma_start(
                        out=qwB[:, o2 * 512:(o2 + 1) * 512],
                        in_=rep_src(qw_o_d[:], ot0 + o2, 0, NHC, NHC))
                scB = qp.tile([128, 8 * 128], F16, tag="scB")
                nc.sync.dma_start(out=scB[:, 0:1024],
                                  in_=sc_src(sc_o_d[:], ot0, 0, 8, NHC))
                nib = qn.tile([128, 8 * 128], I32, tag="nib")
                nc.vector.tensor_scalar(
                    out=nib[:, 0:1024], in0=qwB[:, 0:1024],
                    scalar1=shc[:], scalar2=15,
                    op0=Alu.logical_shift_right, op1=Alu.bitwise_and)
                nc.vector.tensor_tensor(
                    out=w16[:, 0:1024], in0=nib[:, 0:1024],
                    in1=scB[:, 0:1024], op=Alu.mult)
                zl = qp.tile([32, 256], F16, tag="zl")
                nc.sync.dma_start(
                    out=zl[0:NHC, :],
                    in_=nz_o_d[0:NHC, ot0 * 128:(ot0 + 2) * 128])
                for o2 in range(2):
                    ot = ot0 + o2
                    for ch2 in range(2):
                        cc = ch2 * 512
                        mm = pp.tile([128, 512], F32, tag="mm")
                        for kt in range(NHC):
                            nc.tensor.matmul(
                                mm[:],
                                w16[:, o2 * 512 + kt * 128: o2 * 512 + (kt + 1) * 128],
                                oTh[:, kt * SH + cc: kt * SH + cc + 512],
                                start=(kt == 0), stop=False)
                        nc.tensor.matmul(mm[:], zl[0:NHC, o2 * 128:(o2 + 1) * 128],
                                         rsoh[0:NHC, cc:cc + 512],
                                         start=False, stop=True)
                        pt = f16t()
                        nc.scalar.activation(pt[:], mm[:], Act.Copy)
                        nc.scalar.dma_start(
                            out=part1_d[sh][ot * 128:(ot + 1) * 128, cc:cc + 512],
                            in_=pt[:])
            nc.gpsimd.collective_compute(
                "AllReduce", Alu.add,
                replica_groups=[list(range(NCORES))],
                ins=[part1_d[sh][:].opt()], outs=[ar1_d[sh][:].opt()])
        close_pool("hres")
        close_pool("oTr")
        close_pool("oTp")
        close_pool("qk1")
        close_pool("qk2")
        close_pool("xph")
        for u in pre_sched[4]:
            predeq_unit(u)

        # ====================== MLP super-phase =============================
        # (weights were pre-dequantized to DRAM, interleaved with attention)
        xp2 = open_pool(name="xph2", bufs=1)
        x2Tc = xp2.tile([128, NKT * 512], BF16, tag="x2Tc")
        yTc = xp2.tile([128, NIT * 512], BF16, tag="yTc")
        h2p = open_pool(name="h2res", bufs=1)
        h2s = open_pool(name="h2strip", bufs=2)

        for ch in range(NCHUNK):
            c0, c1 = ch * 512, (ch + 1) * 512
            arck = ar1_d[ch // 2]
            ac = (ch % 2) * 512
            # ---- fused h2 = h + ar1 (SBUF resident, f16) + rms2 ----
            h2bf = h2p.tile([128, NKT * 512], F16, tag="h2bf")
            rsx2c = w2.tile([32, 512], BF16, tag="rsx2c", name="rsx2c")
            rsyc = w2.tile([32, 512], BF16, tag="rsyc", name="rsyc")
            ssq_ps = prs.tile([32, 512], F32, tag="rs")
            for st in range(8):
                hst = h2s.tile([128, 4 * 512], F32, tag="hst2", name="hst2")
                nc.gpsimd.dma_start(
                    out=hst[:].rearrange("p (kt s) -> p kt s", s=512),
                    in_=hT_d[st * 512:(st + 1) * 512, c0:c1]
                    .rearrange("(kt p) s -> p kt s", p=128))
                ast = h2s.tile([128, 4 * 512], F16, tag="ast", name="ast")
                nc.sync.dma_start(
                    out=ast[:].rearrange("p (kt s) -> p kt s", s=512),
                    in_=arck[st * 512:(st + 1) * 512, ac:ac + 512]
                    .rearrange("(kt p) s -> p kt s", p=128))
                for k2 in range(4):
                    kt = st * 4 + k2
                    sl2 = slice(k2 * 512, (k2 + 1) * 512)
                    sl = slice(kt * 512, (kt + 1) * 512)
                    nc.vector.tensor_tensor(out=h2bf[:, sl], in0=hst[:, sl2],
                                            in1=ast[:, sl2], op=Alu.add)
                    sq = f16t()
                    nc.scalar.activation(sq[:], h2bf[:, sl], Act.Square)
                    nc.tensor.matmul(ssq_ps[0:1, :], ones16[:], sq[:],
                                     start=(kt == 0), stop=(kt == NKT - 1))
            trow = w3.tile([1, 512], F32, tag="rows")
            nc.vector.tensor_scalar(out=trow[:], in0=ssq_ps[0:1, :],
                                    scalar1=1.0 / HID, scalar2=EPS,
                                    op0=Alu.mult, op1=Alu.add)
            rrow = w3.tile([1, 512], F32, tag="rows")
            nc.vector.reciprocal(rrow[:], trow[:])
            srow = w3.tile([1, 512], F16, tag="rowsh")
            nc.scalar.activation(srow[:], rrow[:], Act.Sqrt)
            rstdB = w2.tile([128, 512], F16, tag="rstdB")
            nc.gpsimd.partition_broadcast(rstdB[:], srow[:])
            rsx_ps = prs.tile([32, 512], F32, tag="rs")
            for kt in range(NKT):
                xsl = x2Tc[:, kt * 512:(kt + 1) * 512]
                nc.vector.scalar_tensor_tensor(
                    out=xsl, in0=h2bf[:, kt * 512:(kt + 1) * 512],
                    scalar=lnw[:, NKT + kt:NKT + kt + 1],
                    in1=rstdB[:], op0=Alu.mult, op1=Alu.mult)
                nc.tensor.matmul(rsx_ps[:], eswbf[:, 31 - kt:63 - kt],
                                 xsl, start=(kt == 0), stop=(kt == NKT - 1))
            nc.scalar.activation(rsx2c[:], rsx_ps[:], Act.Copy)
            # gate/up -> yTc
            for it in range(NIT):
                wg = qp.tile([128, NKT * 128], BF16, tag="w16")
                nc.scalar.dma_start(out=wg[:],
                                    in_=wgu_dd[it * 128:(it + 1) * 128, :])
                zlg = load_zl(nz_gu_d[:], it, 32, BF16)
                wu = qp.tile([128, NKT * 128], BF16, tag="w16")
                nc.scalar.dma_start(
                    out=wu[:],
                    in_=wgu_dd[(NIT + it) * 128:(NIT + it + 1) * 128, :])
                zlu = load_zl(nz_gu_d[:], NIT + it, 32, BF16)
                gp = pp.tile([128, 512], F32, tag="mm")
                for kt in range(NKT):
                    nc.tensor.matmul(
                        gp[:], wg[:, kt * 128:(kt + 1) * 128],
                        x2Tc[:, kt * 512:(kt + 1) * 512],
                        start=(kt == 0), stop=False)
                nc.tensor.matmul(gp[:], zlg[0:32, :], rsx2c[:],
                                 start=False, stop=True)
                up = pp.tile([128, 512], F32, tag="mm")
                for kt in range(NKT):
                    nc.tensor.matmul(
                        up[:], wu[:, kt * 128:(kt + 1) * 128],
                        x2Tc[:, kt * 512:(kt + 1) * 512],
                        start=(kt == 0), stop=False)
                nc.tensor.matmul(up[:], zlu[0:32, :], rsx2c[:],
                                 start=False, stop=True)
                sg = f32t()
                nc.scalar.activation(sg[:], gp[:], Act.Silu)
                nc.vector.tensor_tensor(
                    out=yTc[:, it * 512:(it + 1) * 512],
                    in0=sg[:], in1=up[:], op=Alu.mult)
            # rowsums of yTc
            rs_ps = prs.tile([32, 512], F32, tag="rs")
            for kt in range(NIT):
                nc.tensor.matmul(rs_ps[:], eswbf[:, 31 - kt:63 - kt],
                                 yTc[:, kt * 512:(kt + 1) * 512],
                                 start=(kt == 0), stop=(kt == NIT - 1))
            nc.scalar.activation(rsyc[:], rs_ps[:], Act.Copy)
            # down partials + h2/8 -> part2{a,b}_d
            for ot in range(NKT):
                w16 = qp.tile([128, NKT * 128], BF16, tag="w16")
                nc.scalar.dma_start(out=w16[:, 0:NIT * 128],
                                    in_=wdn_dd[ot * 128:(ot + 1) * 128, :])
                zl = load_zl(nz_dn_d[:], ot, NIT, BF16)
                mm = pp.tile([128, 512], F32, tag="mm")
                for kt in range(NIT):
                    nc.tensor.matmul(
                        mm[:], w16[:, kt * 128:(kt + 1) * 128],
                        yTc[:, kt * 512:(kt + 1) * 512],
                        start=(kt == 0), stop=False)
                nc.tensor.matmul(mm[:], zl[0:NIT, :], rsyc[0:NIT, :],
                                 start=False, stop=True)
                pt = f16t()
                nc.vector.scalar_tensor_tensor(
                    out=pt[:], in0=h2bf[:, ot * 512:(ot + 1) * 512],
                    scalar=1.0 / NCORES, in1=mm[:],
                    op0=Alu.mult, op1=Alu.add)
                dst_d = part2a_d if ot < HKT else part2b_d
                ro = (ot % HKT) * 128
                nc.scalar.dma_start(out=dst_d[ro:ro + 128, c0:c1], in_=pt[:])
        close_pool("h2strip")
        close_pool("h2res")
        close_pool("xph2")

        # =========== exchange 2: 2x half ReduceScatter -> out ===========
        nc.gpsimd.collective_compute(
            "ReduceScatter", Alu.add,
            replica_groups=[list(range(NCORES))],
            ins=[part2a_d[:].opt()], outs=[rs2a_d[:].opt()])
        nc.gpsimd.collective_compute(
            "ReduceScatter", Alu.add,
            replica_groups=[list(range(NCORES))],
            ins=[part2b_d[:].opt()], outs=[rs2b_d[:].opt()])
        nc.sync.dma_start(out=outA_d[:], in_=rs2a_d[:])
        nc.sync.dma_start(out=outB_d[:], in_=rs2b_d[:])

        for cm, nm in reversed(ctx_pools):
            cm.__exit__(None, None, None)
        ctx_pools.clear()

    nc.compile()
    return nc


def _host_prep(inputs):
    """Build the 8 per-core input maps from full inputs."""
    import ml_dtypes
    bf16 = ml_dtypes.bfloat16
    f16 = np.float16

    def unpack_z1(qz):
        sh = (np.arange(8, dtype=np.uint32) * 4)
        z = ((qz[:, :, None].view(np.uint32) >> sh[None, None, :]) & 15)
        return z.reshape(qz.shape[0], -1).astype(np.float32) + 1.0

    h = np.asarray(inputs["hidden_states"], np.float32)[0]     # [S, HID]
    hT = np.ascontiguousarray(h.T)                             # [HID, S]
    sin = np.asarray(inputs["sin"], np.float32)                # [S, HD]
    cos = np.asarray(inputs["cos"], np.float32)
    cosT = np.ascontiguousarray(cos.T).astype(f16)
    sinf = sin.T.copy()
    sinf[0:64, :] *= -1.0                                      # rot-half sign fold
    sinfT = np.ascontiguousarray(sinf).astype(f16)

    qkv_qw, qkv_sc, qkv_nz = [], [], []
    for nm in ("q", "k", "v"):
        qw = np.asarray(inputs["qw_" + nm])
        sc = np.asarray(inputs["sc_" + nm], np.float32)
        z1 = unpack_z1(np.asarray(inputs["qz_" + nm]))
        qkv_qw.append(qw); qkv_sc.append(sc); qkv_nz.append(-(z1 * sc))

    qw_o = np.asarray(inputs["qw_o"])
    sc_o = np.asarray(inputs["sc_o"], np.float32)
    nz_o = -(unpack_z1(np.asarray(inputs["qz_o"])) * sc_o)

    def pad_cols(a, w):
        out = np.zeros((a.shape[0], w), a.dtype)
        out[:, :a.shape[1]] = a
        return out

    qw_g = pad_cols(np.asarray(inputs["qw_gate"]), IPAD)
    qw_u = pad_cols(np.asarray(inputs["qw_up"]), IPAD)
    sc_g = pad_cols(np.asarray(inputs["sc_gate"], np.float32), IPAD)
    sc_u = pad_cols(np.asarray(inputs["sc_up"], np.float32), IPAD)
    nz_g = pad_cols(-(unpack_z1(np.asarray(inputs["qz_gate"]))
                      * np.asarray(inputs["sc_gate"], np.float32)), IPAD)
    nz_u = pad_cols(-(unpack_z1(np.asarray(inputs["qz_up"]))
                      * np.asarray(inputs["sc_up"], np.float32)), IPAD)

    qw_dn = np.zeros((IPAD // 8, HID), np.int32)
    qw_dn[:INTER // 8] = np.asarray(inputs["qw_down"])
    sc_dn = np.zeros((IPAD // GS, HID), np.float32)
    sc_dn[:INTER // GS] = np.asarray(inputs["sc_down"], np.float32)
    nz_dn = np.zeros((IPAD // GS, HID), np.float32)
    nz_dn[:INTER // GS] = -(unpack_z1(np.asarray(inputs["qz_down"]))
                            * np.asarray(inputs["sc_down"], np.float32))

    ln1 = np.asarray(inputs["ln1_w"], np.float32)
    ln2 = np.asarray(inputs["ln2_w"], np.float32)

    def repack_qw(qw):
        # [nkt*16, notile*128] -> [notile*16, nkt*128], rows o*16+r,
        # k-tiles contiguous per out-tile
        nkt = qw.shape[0] // 16
        notile = qw.shape[1] // 128
        return np.ascontiguousarray(
            qw.reshape(nkt, 16, notile, 128).transpose(2, 1, 0, 3)
            .reshape(notile * 16, nkt * 128))

    def repack_sc(sc):
        # [G, notile*128] -> [notile*G, 128], rows o*G+g
        G = sc.shape[0]
        notile = sc.shape[1] // 128
        return np.ascontiguousarray(
            sc.reshape(G, notile, 128).transpose(1, 0, 2)
            .reshape(notile * G, 128))

    maps = []
    for c in range(NCORES):
        cs = slice(c * OPC, (c + 1) * OPC)
        isl = slice(c * IPC, (c + 1) * IPC)
        m = {
            "hT": hT, "cosT": cosT, "sinfT": sinfT, "ln1": ln1, "ln2": ln2,
            "qw_qkv": repack_qw(
                np.concatenate([qkv_qw[i][:, cs] for i in range(3)], axis=1)),
            "sc_qkv": repack_sc(
                np.concatenate([qkv_sc[i][:, cs] for i in range(3)],
                               axis=1).astype(f16)),
            "nz_qkv": np.ascontiguousarray(
                np.concatenate([qkv_nz[i][:, cs] for i in range(3)],
                               axis=1)).astype(f16),
            "qw_o": repack_qw(qw_o[c * OPC // 8:(c + 1) * OPC // 8]),
            "sc_o": repack_sc(
                sc_o[c * OPC // GS:(c + 1) * OPC // GS].astype(f16)),
            "nz_o": np.ascontiguousarray(
                nz_o[c * OPC // GS:(c + 1) * OPC // GS]).astype(f16),
            "qw_gu": repack_qw(
                np.concatenate([qw_g[:, isl], qw_u[:, isl]], axis=1)),
            "sc_gu": repack_sc(
                np.concatenate([sc_g[:, isl], sc_u[:, isl]],
                               axis=1).astype(bf16)),
            "nz_gu": np.ascontiguousarray(
                np.concatenate([nz_g[:, isl], nz_u[:, isl]],
                               axis=1)).astype(bf16),
            "qw_dn": repack_qw(qw_dn[c * IPC // 8:(c + 1) * IPC // 8]),
            "sc_dn": repack_sc(
                sc_dn[c * NIT:(c + 1) * NIT].astype(bf16)),
            "nz_dn": np.ascontiguousarray(
                nz_dn[c * NIT:(c + 1) * NIT]).astype(bf16),
        }
        maps.append(m)
    return maps


def run(inputs, trace=False):
    from concourse.bass_utils import run_bass_kernel_spmd
    if "rel" not in _BUILD_CACHE:
        _BUILD_CACHE["rel"] = _build()
    nc = _BUILD_CACHE["rel"]
    maps = _host_prep(inputs)
    res = run_bass_kernel_spmd(nc, maps, core_ids=list(range(NCORES)),
                               trace=trace)
    HO = OPC // 2
    outT = np.empty((HID, S), np.float32)
    for c in range(NCORES):
        outT[c * HO:(c + 1) * HO] = res.results[c]["outA"]
        outT[HID // 2 + c * HO: HID // 2 + (c + 1) * HO] = res.results[c]["outB"]
    out = np.ascontiguousarray(outT.T)[None]
    return out, res


def kernel(**inputs):
    out, _ = run(inputs)
    return out
